# revision 1
# baseline (speedup 1.0000x reference)
"""TRN2 Bass kernel for nn_DebateModel (v1 hybrid).

Device (8 NeuronCores, data-parallel over comments, 8 comments/core):
streams the full token_embed (the memory-dominant input, 201 MB) through
the bidirectional span-encoder input projections
    xp_d = W_ih_d @ x^T   for d in {fwd, bwd}   (fp16 operands, fp32 psum)
which is the bulk of the model's FLOPs and memory traffic.

Host: the sequential LSTM recurrences (latency-bound on TRN2's engines),
span gathers, the per-comment GAT/attention head and the comment
compressor, in fp32 numpy, consuming the device-computed projections.

Self-contained: hardcodes all shapes; no sibling imports.
"""
import sys
import numpy as np

sys.path.insert(0, '/opt/trn_rl_repo')

C, L, FEAT = 64, 1024, 768
H = 80
SPAN = 4 * H            # 320
GATES = 4 * H           # 320 per direction
N_CORES = 8
CPC = C // N_CORES      # comments per core = 8
TOK = CPC * L           # tokens per core = 8192
KCH = FEAT // 128       # 6 contraction chunks
GCH = (2 * GATES) // 128  # 5 gate chunks over both directions (640)
TBLK = 512              # moving-operand token block
NTB = TOK // TBLK       # 16

_compiled = None


def _build():
    import concourse.bass as bass
    import concourse.tile as tile
    from concourse import bacc, mybir
    from contextlib import ExitStack

    f16, f32 = mybir.dt.float16, mybir.dt.float32

    nc = bacc.Bacc("TRN2", target_bir_lowering=False, debug=False,
                   enable_asserts=False, num_devices=N_CORES)

    xt_d = nc.dram_tensor("xt", [KCH, 128, TOK], f16, kind="ExternalInput").ap()
    w_d = nc.dram_tensor("w", [KCH, GCH, 128, 128], f16,
                         kind="ExternalInput").ap()
    xp_d = nc.dram_tensor("xp", [GCH, 128, TOK], f32,
                          kind="ExternalOutput").ap()

    with tile.TileContext(nc) as tc, ExitStack() as ctx:
        wpool = ctx.enter_context(tc.tile_pool(name="w", bufs=1))
        xpool = ctx.enter_context(tc.tile_pool(name="x", bufs=3))
        opool = ctx.enter_context(tc.tile_pool(name="o", bufs=3))
        ppool = ctx.enter_context(tc.tile_pool(name="p", bufs=2, space="PSUM"))

        wt = wpool.tile([128, KCH * GCH * 128], f16)
        wt3 = {}
        for k in range(KCH):
            for g in range(GCH):
                wt3[k, g] = wt[:, bass.ts(k * GCH + g, 128)]
                nc.sync.dma_start(wt3[k, g], w_d[k, g])

        for tb in range(NTB):
            xts = []
            for k in range(KCH):
                xtile = xpool.tile([128, TBLK], f16, tag=f"x{k}")
                nc.sync.dma_start(xtile[:], xt_d[k, :, bass.ts(tb, TBLK)])
                xts.append(xtile)
            for g in range(GCH):
                ps = ppool.tile([128, TBLK], f32, tag="ps")
                for k in range(KCH):
                    nc.tensor.matmul(ps[:], wt3[k, g], xts[k][:],
                                     start=(k == 0), stop=(k == KCH - 1))
                ot = opool.tile([128, TBLK], f32, tag="ot")
                nc.scalar.copy(ot[:], ps[:])
                nc.sync.dma_start(xp_d[g, :, bass.ts(tb, TBLK)], ot[:])
    nc.compile()
    return nc


def _sigmoid(z):
    out = np.empty_like(z)
    np.negative(z, out)
    np.exp(out, out)
    out += 1.0
    np.reciprocal(out, out)
    return out


def _lstm(xp, Whh, b, reverse=False):
    """xp: [L, B, 320] precomputed x @ Wih.T. Returns hidden states
    [L, B, 80] fp32, exact fp32 serial recurrence."""
    Ln, B, _ = xp.shape
    Wt = Whh.T.astype(np.float32)
    h = np.zeros((B, H), np.float32)
    c = np.zeros((B, H), np.float32)
    hs = np.empty((Ln, B, H), np.float32)
    order = range(Ln - 1, -1, -1) if reverse else range(Ln)
    for t in order:
        z = xp[t] + h @ Wt + b
        i, f, g, o = z[:, :H], z[:, H:2*H], z[:, 2*H:3*H], z[:, 3*H:]
        c = _sigmoid(f) * c + _sigmoid(i) * np.tanh(g)
        h = _sigmoid(o) * np.tanh(c)
        hs[t] = h
    return hs


def _attn_pool(feats, vals, mask, W1, b1, W2, b2):
    s = np.maximum(feats @ W1 + b1, 0.0) @ W2 + b2
    s = np.where(mask[:, None], s, -1e9)
    ex = np.exp(s - s.max(0, keepdims=True))
    a = ex / ex.sum(0, keepdims=True)
    a = np.where(mask[:, None], a, 0.0)
    out = (a * vals).sum(0)
    return np.where(mask.any(), out, np.zeros_like(out))


def _gat(h, src, dst, emask, Wm, a_l, a_r, bias):
    An, K = h.shape[0], Wm.shape[0]
    hp = np.stack([h @ Wm[k] for k in range(K)], 1)          # [A, K, D]
    el = (hp * a_l[None]).sum(-1)
    er = (hp * a_r[None]).sum(-1)
    e = el[src] + er[dst]
    e = np.where(e > 0, e, 0.2 * e)
    e = np.where(emask[:, None], e, -1e9)
    m = np.full((An, K), -1e9, np.float32)
    np.maximum.at(m, dst, e)
    ex = np.where(emask[:, None], np.exp(e - m[dst]), 0.0)
    den = np.zeros((An, K), np.float32)
    np.add.at(den, dst, ex)
    alpha = ex / np.maximum(den[dst], 1e-9)
    out = np.zeros((An, K, hp.shape[2]), np.float32)
    np.add.at(out, dst, alpha[:, :, None] * hp[src])
    out = out + bias[None]
    out = np.where(out > 0, out, np.expm1(np.minimum(out, 0.0)))
    return out.reshape(An, -1)


def kernel(**inputs):
    global _compiled
    inp = {k: np.asarray(v) for k, v in inputs.items()}

    # ---- device: input projections over all tokens ----
    token = inp['token_embed'].astype(np.float32)            # [C, L, 768]
    Wih2 = np.concatenate([inp['Wih_f'], inp['Wih_b']], 0)   # [640, 768]
    # pack stationary chunks: w[k, g, p, q] = Wih2[g*128+q, k*128+p]
    wpk = np.ascontiguousarray(
        Wih2.reshape(GCH, 128, KCH, 128).transpose(2, 0, 3, 1)
    ).astype(np.float16)                                     # [6, 5, 128, 128]

    in_maps = []
    for core in range(N_CORES):
        tk = token[core*CPC:(core+1)*CPC]                     # [8, 1024, 768]
        xt = np.ascontiguousarray(
            tk.reshape(TOK, KCH, 128).transpose(1, 2, 0)
        ).astype(np.float16)                                  # [6, 128, 8192]
        in_maps.append(dict(xt=xt, w=wpk))

    if _compiled is None:
        _compiled = _build()
    globals()['_last_in_maps'] = in_maps
    from concourse.bass_utils import run_bass_kernel_spmd
    import time as _time
    _t0 = _time.time()
    res = run_bass_kernel_spmd(_compiled, in_maps,
                               core_ids=list(range(N_CORES)))
    globals()['_last_exec_ns'] = res.exec_time_ns
    globals()['_last_dispatch_s'] = _time.time() - _t0

    xp_all = np.empty((C, L, 2 * GATES), np.float32)
    for core in range(N_CORES):
        xpc = res.results[core]["xp"]                         # [5, 128, 8192]
        xpc = xpc.reshape(2 * GATES, CPC, L).transpose(1, 2, 0)
        xp_all[core*CPC:(core+1)*CPC] = xpc

    # ---- host: recurrences + heads (fp32) ----
    xp_f = np.ascontiguousarray(
        xp_all[:, :, :GATES].transpose(1, 0, 2)) + inp['b_f']  # [L, C, 320]
    xp_b = np.ascontiguousarray(
        xp_all[:, :, GATES:].transpose(1, 0, 2)) + inp['b_b']
    hf = _lstm(xp_f, inp['Whh_f'], 0.0).transpose(1, 0, 2)     # [C, L, 80]
    hb = _lstm(xp_b, inp['Whh_b'], 0.0, reverse=True).transpose(1, 0, 2)

    A = inp['adu_spans'].shape[1]
    W_gat = inp['W_gat'].astype(np.float32)

    def span_rep(c, spans):
        i, j = spans[..., 0], spans[..., 1]
        return np.concatenate([hf[c][j] - hf[c][i - 1], hb[c][i] - hb[c][j + 1],
                               hf[c][i - 1], hb[c][j + 1]], -1)

    rows = []
    for c in range(C):
        cemb = span_rep(c, inp['comment_spans'][c])
        amask = inp['adu_masks'][c]
        adus = span_rep(c, inp['adu_spans'][c]) * amask[:, None]
        isrc, idst = inp['inner_src'][c], inp['inner_dst'][c]
        irel, imask = inp['inner_rel'][c], inp['inner_mask'][c]
        tsrc, tdst = inp['inter_src'][c], inp['inter_dst'][c]
        trel, tmask = inp['inter_rel'][c], inp['inter_mask'][c]
        srcs = [isrc, isrc, tdst, tdst]
        dsts = [idst, idst, tsrc, tsrc]
        masks = [imask & (irel == 0), imask & (irel == 1),
                 tmask & (trel == 0), tmask & (trel == 1)]
        z = np.stack([_gat(adus, srcs[m], dsts[m], masks[m], W_gat[m],
                           inp['a_l'][m], inp['a_r'][m], inp['b_gat'][m])
                      for m in range(4)])                     # [4, A, 768]
        w = np.tanh(z.reshape(4 * A, -1) @ inp['W_sem'] + inp['b_sem'])
        w = (w @ inp['q_sem']).reshape(4, A)
        w = (w * amask[None]).sum(1) / max(amask.sum(), 1)
        beta = np.exp(w - w.max())
        beta /= beta.sum()
        zfin = np.einsum('m,mad->ad', beta, z)
        adu_embeds = zfin @ inp['W_pred'] + inp['b_pred']
        feats = np.concatenate(
            [np.broadcast_to(cemb, (A, SPAN)), adu_embeds], -1)
        att_adu = _attn_pool(feats, adu_embeds, amask & inp['local_masks'][c],
                             inp['W_adu1'], inp['b_adu1'],
                             inp['W_adu2'], inp['b_adu2'])

        def pair(se, de, rel, me, W1, b1, W2, b2):
            onehot = np.stack([rel, 1 - rel], -1).astype(np.float32)
            pe = np.concatenate([adu_embeds[se], adu_embeds[de], onehot], -1)
            fp = np.concatenate(
                [np.broadcast_to(cemb, (pe.shape[0], SPAN)), pe], -1)
            return _attn_pool(fp, pe, me, W1, b1, W2, b2)

        att_inn = pair(isrc, idst, irel, imask, inp['W_inn1'], inp['b_inn1'],
                       inp['W_inn2'], inp['b_inn2'])
        att_int = pair(tdst, tsrc, trel, tmask, inp['W_int1'], inp['b_int1'],
                       inp['W_int2'], inp['b_int2'])
        rows.append(np.concatenate(
            [att_adu, att_inn, att_int, inp['info_scores'][c], cemb]))
    wo_ctx = np.stack(rows).astype(np.float32)                # [64, 1608]

    xpc = (wo_ctx @ inp['Wih_c'].T + inp['b_c'])[:, None, :]  # [64, 1, 800]
    globals()['H'], hs = 200, None
    try:
        hs = _lstm(xpc, inp['Whh_c'], 0.0)[:, 0, :]           # [64, 200]
    finally:
        globals()['H'] = 80
    return np.concatenate([hs, wo_ctx], -1).astype(np.float32)



# revision 2
# speedup vs baseline: 3.7544x; 3.7544x over previous
"""TRN2 Bass kernel for nn_DebateModel (v2: on-device BiLSTM).

Device (8 NeuronCores, data-parallel over comments, 8 comments/core):
 - input projections xp = W_ih @ x for both directions (bulk of FLOPs)
 - the full bidirectional LSTM recurrence (1024 coupled fwd/bwd steps)
 - returns only the hidden states [80, 1024, 16] fp16 (2.6 MB/core)

Host: span gathers, per-comment GAT/attention head, comment compressor
(fp32 numpy) — cheap graph math on tiny [32..48]-sized tensors.

Layouts (per core, transposed: gate/hidden dim on partitions):
 - xt   [6, 128, 8192]  tokens, K-chunked; token n = t*8 + c (t-major)
 - wih  [6, 8, 128, 80] stationary chunks; gd = gate*2 + dir,
                        gate order [i, f, o, g] (torch rows 0/80/240/160)
 - whh  [8, 80, 80]     recurrent stationary per gd
 - bias [80, 8]         per-gate bias columns
 - xp SBUF [80, 1024, 4, 16]: per step 64 cols = 4 gates x (8 fwd, 8 bwd)
 - ht SBUF [80, 1025, 16]: step k writes k+1; fwd cols 0:8 = position k,
   bwd cols 8:16 = position 1023-k.

Self-contained: hardcodes all shapes; no sibling imports.
"""
import sys
import numpy as np

sys.path.insert(0, '/opt/trn_rl_repo')

C, L, FEAT = 64, 1024, 768
H = 80
SPAN = 4 * H            # 320
N_CORES = 8
CPC = C // N_CORES      # comments per core = 8
TOK = CPC * L           # tokens per core = 8192
KCH = FEAT // 128       # 6 contraction chunks
TBLK = 512              # projection token block (64 steps)
NTB = TOK // TBLK       # 16
GD = 8                  # gate-dir count
ROWBASE = [0, 80, 240, 160]   # i, f, o, g -> torch row offset

TOK_FP8 = False         # token/wih upload dtype switch

_compiled = None

# Warm the axon/jax platform at import time (device discovery is a
# one-time global cost; keep it out of the compute path).
try:
    import jax as _jax
    _jax.devices()
except Exception:
    pass


def _build():
    import concourse.bass as bass
    import concourse.tile as tile
    from concourse import bacc, mybir
    from contextlib import ExitStack

    f16, f32 = mybir.dt.float16, mybir.dt.float32
    tdt = mybir.dt.float8e4 if TOK_FP8 else f16
    ACT = mybir.ActivationFunctionType

    nc = bacc.Bacc("TRN2", target_bir_lowering=False, debug=False,
                   enable_asserts=False, num_devices=N_CORES)

    xt_d = nc.dram_tensor("xt", [KCH, 128, TOK], tdt, kind="ExternalInput").ap()
    wih_d = nc.dram_tensor("wih", [KCH, GD, 128, H], tdt,
                           kind="ExternalInput").ap()
    whh_d = nc.dram_tensor("whh", [GD, H, H], f16, kind="ExternalInput").ap()
    b_d = nc.dram_tensor("b", [H, GD], f32, kind="ExternalInput").ap()
    ht_d = nc.dram_tensor("ht", [H, L, 2 * CPC], f16,
                          kind="ExternalOutput").ap()

    with tile.TileContext(nc) as tc, ExitStack() as ctx:
        state = ctx.enter_context(tc.tile_pool(name="st", bufs=1))
        xpool = ctx.enter_context(tc.tile_pool(name="x", bufs=2))
        gpool = ctx.enter_context(tc.tile_pool(name="g", bufs=2))
        ppool = ctx.enter_context(tc.tile_pool(name="p", bufs=4, space="PSUM"))

        # --- persistent tiles ---
        wih = state.tile([128, KCH * GD * H], tdt, tag="wih")
        for k in range(KCH):
            for gd in range(GD):
                nc.sync.dma_start(wih[:, (k * GD + gd) * H:(k * GD + gd + 1) * H],
                                  wih_d[k, gd])
        whh = state.tile([H, GD * H], f16, tag="whh")
        for gd in range(GD):
            nc.sync.dma_start(whh[:, gd * H:(gd + 1) * H], whh_d[gd])
        bias = state.tile([H, GD], f32, tag="bias")
        nc.sync.dma_start(bias[:], b_d[:])

        xp = state.tile([H, L, 4, 2 * CPC], f16, tag="xp")
        ht = state.tile([H, L + 1, 2 * CPC], f16, tag="ht")
        cst = state.tile([H, 2 * CPC], f32, tag="c")
        nc.vector.memset(ht[:, 0, :], 0.0)
        nc.vector.memset(cst[:], 0.0)

        # --- phase 1: input projections ---
        for tb in range(NTB):
            xts = []
            for k in range(KCH):
                xtile = xpool.tile([128, TBLK], tdt, tag=f"x{k}")
                nc.sync.dma_start(xtile[:], xt_d[k, :, tb * TBLK:(tb + 1) * TBLK])
                xts.append(xtile)
            for g in range(4):
                for d in range(2):
                    gd = g * 2 + d
                    ps = ppool.tile([H, TBLK // CPC, CPC], f32, tag="ps")
                    for k in range(KCH):
                        nc.tensor.matmul(
                            ps[:], wih[:, (k * GD + gd) * H:(k * GD + gd + 1) * H],
                            xts[k][:], start=(k == 0), stop=(k == KCH - 1))
                    # xp[:, steps, g, d*8:(d+1)*8] = ps + b[gd]
                    nc.scalar.activation(
                        xp[:, tb * (TBLK // CPC):(tb + 1) * (TBLK // CPC),
                           g, d * CPC:(d + 1) * CPC],
                        ps[:], ACT.Identity, bias=bias[:, gd:gd + 1])

        # --- phase 2: coupled fwd/bwd recurrence ---
        for k in range(L):
            pg = ppool.tile([H, 4, 2 * CPC], f32, tag="pg")
            for g in range(4):
                for d in range(2):
                    gd = g * 2 + d
                    nc.tensor.matmul(
                        pg[:, g, d * CPC:(d + 1) * CPC],
                        whh[:, gd * H:(gd + 1) * H],
                        ht[:, k, d * CPC:(d + 1) * CPC],
                        start=True, stop=True)
            gates = gpool.tile([H, 4, 2 * CPC], f32, tag="gates")
            sg = gpool.tile([H, 4, 2 * CPC], f32, tag="sg")
            th = gpool.tile([H, 2 * CPC], f32, tag="th")
            # z = pg + xp (fwd reads step k, bwd reads step 1023-k)
            nc.vector.tensor_add(gates[:, :, 0:CPC], pg[:, :, 0:CPC],
                                 xp[:, k, :, 0:CPC])
            nc.vector.tensor_add(gates[:, :, CPC:2 * CPC], pg[:, :, CPC:2 * CPC],
                                 xp[:, L - 1 - k, :, CPC:2 * CPC])
            nc.scalar.activation(sg[:, 0:3, :], gates[:, 0:3, :], ACT.Sigmoid)
            nc.scalar.activation(sg[:, 3, :], gates[:, 3, :], ACT.Tanh)
            # c = sig(f)*c + sig(i)*tanh(g)
            u = gpool.tile([H, 2 * CPC], f32, tag="u")
            fc = gpool.tile([H, 2 * CPC], f32, tag="fc")
            nc.vector.tensor_mul(u[:], sg[:, 0, :], sg[:, 3, :])
            nc.vector.tensor_mul(fc[:], sg[:, 1, :], cst[:])
            nc.vector.tensor_add(cst[:], fc[:], u[:])
            nc.scalar.activation(th[:], cst[:], ACT.Tanh)
            # h = sig(o)*tanh(c) -> f16 history (next step's moving operand)
            nc.vector.tensor_mul(ht[:, k + 1, :], sg[:, 2, :], th[:])

        # --- output ---
        nc.sync.dma_start(ht_d[:], ht[:, 1:L + 1, :])
    nc.compile()
    return nc


def _np_tdt():
    from concourse import mybir
    return mybir.dt.np(mybir.dt.float8e4) if TOK_FP8 else np.float16


def _pack_inputs(inp):
    tdt = _np_tdt()
    token = inp['token_embed']                                # [C, L, 768] f32

    wih_p = np.empty((KCH, GD, 128, H), np.float32)
    whh_p = np.empty((GD, H, H), np.float32)
    b_p = np.empty((H, GD), np.float32)
    for g in range(4):
        rb = ROWBASE[g]
        for d in range(2):
            Wih = inp['Wih_f'] if d == 0 else inp['Wih_b']    # [320, 768]
            Whh = inp['Whh_f'] if d == 0 else inp['Whh_b']    # [320, 80]
            bb = inp['b_f'] if d == 0 else inp['b_b']         # [320]
            gd = g * 2 + d
            wih_p[:, gd] = Wih[rb:rb + H].reshape(H, KCH, 128).transpose(1, 2, 0)
            whh_p[gd] = Whh[rb:rb + H].T
            b_p[:, gd] = bb[rb:rb + H]
    wih_p = wih_p.astype(tdt)
    whh_p = whh_p.astype(np.float16)

    in_maps = []
    for core in range(N_CORES):
        tk = token[core * CPC:(core + 1) * CPC]               # [8, 1024, 768]
        xt = np.ascontiguousarray(
            tk.transpose(1, 0, 2).reshape(TOK, KCH, 128).transpose(1, 2, 0)
        ).astype(tdt)                                         # [6, 128, 8192]
        in_maps.append(dict(xt=xt, wih=wih_p, whh=whh_p, b=b_p))
    return in_maps


def _sigmoid(z):
    out = np.empty_like(z)
    np.negative(z, out)
    np.exp(out, out)
    out += 1.0
    np.reciprocal(out, out)
    return out


def _lstm(xp, Whh, nh, reverse=False):
    """xp: [T, B, 4*nh] precomputed x @ Wih.T + b. Exact fp32 recurrence."""
    Ln, B, _ = xp.shape
    Wt = Whh.T.astype(np.float32)
    h = np.zeros((B, nh), np.float32)
    c = np.zeros((B, nh), np.float32)
    hs = np.empty((Ln, B, nh), np.float32)
    order = range(Ln - 1, -1, -1) if reverse else range(Ln)
    for t in order:
        z = xp[t] + h @ Wt
        i, f, g, o = (z[:, :nh], z[:, nh:2 * nh],
                      z[:, 2 * nh:3 * nh], z[:, 3 * nh:])
        c = _sigmoid(f) * c + _sigmoid(i) * np.tanh(g)
        h = _sigmoid(o) * np.tanh(c)
        hs[t] = h
    return hs


def _attn_pool(feats, vals, mask, W1, b1, W2, b2):
    s = np.maximum(feats @ W1 + b1, 0.0) @ W2 + b2
    s = np.where(mask[:, None], s, -1e9)
    ex = np.exp(s - s.max(0, keepdims=True))
    a = ex / ex.sum(0, keepdims=True)
    a = np.where(mask[:, None], a, 0.0)
    out = (a * vals).sum(0)
    return np.where(mask.any(), out, np.zeros_like(out))


def _gat(h, src, dst, emask, Wm, a_l, a_r, bias):
    An, K = h.shape[0], Wm.shape[0]
    hp = np.stack([h @ Wm[k] for k in range(K)], 1)          # [A, K, D]
    el = (hp * a_l[None]).sum(-1)
    er = (hp * a_r[None]).sum(-1)
    e = el[src] + er[dst]
    e = np.where(e > 0, e, 0.2 * e)
    e = np.where(emask[:, None], e, -1e9)
    m = np.full((An, K), -1e9, np.float32)
    np.maximum.at(m, dst, e)
    ex = np.where(emask[:, None], np.exp(e - m[dst]), 0.0)
    den = np.zeros((An, K), np.float32)
    np.add.at(den, dst, ex)
    alpha = ex / np.maximum(den[dst], 1e-9)
    out = np.zeros((An, K, hp.shape[2]), np.float32)
    np.add.at(out, dst, alpha[:, :, None] * hp[src])
    out = out + bias[None]
    out = np.where(out > 0, out, np.expm1(np.minimum(out, 0.0)))
    return out.reshape(An, -1)


def kernel(**inputs):
    global _compiled
    inp = {k: np.asarray(v) for k, v in inputs.items()}
    in_maps = _pack_inputs(inp)

    if _compiled is None:
        _compiled = _build()
    globals()['_last_in_maps'] = in_maps
    from concourse.bass_utils import run_bass_kernel_spmd
    import time as _time
    _t0 = _time.time()
    res = run_bass_kernel_spmd(_compiled, in_maps,
                               core_ids=list(range(N_CORES)))
    globals()['_last_exec_ns'] = res.exec_time_ns
    globals()['_last_dispatch_s'] = _time.time() - _t0

    # unpack hidden states: ht [80, 1024, 16] f16 per core
    hf = np.empty((C, L, H), np.float32)
    hb = np.empty((C, L, H), np.float32)
    for core in range(N_CORES):
        htc = res.results[core]["ht"].astype(np.float32)      # [80, 1024, 16]
        hf[core * CPC:(core + 1) * CPC] = htc[:, :, 0:CPC].transpose(2, 1, 0)
        hb[core * CPC:(core + 1) * CPC] = \
            htc[:, ::-1, CPC:2 * CPC].transpose(2, 1, 0)

    # ---- host: graph heads (fp32) ----
    A = inp['adu_spans'].shape[1]
    W_gat = inp['W_gat'].astype(np.float32)

    def span_rep(c, spans):
        i, j = spans[..., 0], spans[..., 1]
        return np.concatenate([hf[c][j] - hf[c][i - 1], hb[c][i] - hb[c][j + 1],
                               hf[c][i - 1], hb[c][j + 1]], -1)

    rows = []
    for c in range(C):
        cemb = span_rep(c, inp['comment_spans'][c])
        amask = inp['adu_masks'][c]
        adus = span_rep(c, inp['adu_spans'][c]) * amask[:, None]
        isrc, idst = inp['inner_src'][c], inp['inner_dst'][c]
        irel, imask = inp['inner_rel'][c], inp['inner_mask'][c]
        tsrc, tdst = inp['inter_src'][c], inp['inter_dst'][c]
        trel, tmask = inp['inter_rel'][c], inp['inter_mask'][c]
        srcs = [isrc, isrc, tdst, tdst]
        dsts = [idst, idst, tsrc, tsrc]
        masks = [imask & (irel == 0), imask & (irel == 1),
                 tmask & (trel == 0), tmask & (trel == 1)]
        z = np.stack([_gat(adus, srcs[m], dsts[m], masks[m], W_gat[m],
                           inp['a_l'][m], inp['a_r'][m], inp['b_gat'][m])
                      for m in range(4)])                     # [4, A, 768]
        w = np.tanh(z.reshape(4 * A, -1) @ inp['W_sem'] + inp['b_sem'])
        w = (w @ inp['q_sem']).reshape(4, A)
        w = (w * amask[None]).sum(1) / max(amask.sum(), 1)
        beta = np.exp(w - w.max())
        beta /= beta.sum()
        zfin = np.einsum('m,mad->ad', beta, z)
        adu_embeds = zfin @ inp['W_pred'] + inp['b_pred']
        feats = np.concatenate(
            [np.broadcast_to(cemb, (A, SPAN)), adu_embeds], -1)
        att_adu = _attn_pool(feats, adu_embeds, amask & inp['local_masks'][c],
                             inp['W_adu1'], inp['b_adu1'],
                             inp['W_adu2'], inp['b_adu2'])

        def pair(se, de, rel, me, W1, b1, W2, b2):
            onehot = np.stack([rel, 1 - rel], -1).astype(np.float32)
            pe = np.concatenate([adu_embeds[se], adu_embeds[de], onehot], -1)
            fp = np.concatenate(
                [np.broadcast_to(cemb, (pe.shape[0], SPAN)), pe], -1)
            return _attn_pool(fp, pe, me, W1, b1, W2, b2)

        att_inn = pair(isrc, idst, irel, imask, inp['W_inn1'], inp['b_inn1'],
                       inp['W_inn2'], inp['b_inn2'])
        att_int = pair(tdst, tsrc, trel, tmask, inp['W_int1'], inp['b_int1'],
                       inp['W_int2'], inp['b_int2'])
        rows.append(np.concatenate(
            [att_adu, att_inn, att_int, inp['info_scores'][c], cemb]))
    wo_ctx = np.stack(rows).astype(np.float32)                # [64, 1608]

    xpc = (wo_ctx @ inp['Wih_c'].T + inp['b_c'])[:, None, :]  # [64, 1, 800]
    hs = _lstm(xpc, inp['Whh_c'], 200)[:, 0, :]               # [64, 200]
    return np.concatenate([hs, wo_ctx], -1).astype(np.float32)


# revision 3
# speedup vs baseline: 27.4027x; 7.2989x over previous
"""TRN2 Bass kernel for nn_DebateModel (v2: on-device BiLSTM).

Device (8 NeuronCores, data-parallel over comments, 8 comments/core):
 - input projections xp = W_ih @ x for both directions (bulk of FLOPs)
 - the full bidirectional LSTM recurrence (1024 coupled fwd/bwd steps)
 - returns only the hidden states [80, 1024, 16] fp16 (2.6 MB/core)

Host: span gathers, per-comment GAT/attention head, comment compressor
(fp32 numpy) — cheap graph math on tiny [32..48]-sized tensors.

Layouts (per core, transposed: gate/hidden dim on partitions):
 - xt   [6, 128, 8192]  tokens, K-chunked; token n = t*8 + c (t-major)
 - wih  [6, 8, 128, 80] stationary chunks; gd = gate*2 + dir,
                        gate order [i, f, o, g] (torch rows 0/80/240/160)
 - whh  [8, 80, 80]     recurrent stationary per gd
 - bias [80, 8]         per-gate bias columns
 - xp SBUF [80, 1024, 4, 16]: per step 64 cols = 4 gates x (8 fwd, 8 bwd)
 - ht SBUF [80, 1025, 16]: step k writes k+1; fwd cols 0:8 = position k,
   bwd cols 8:16 = position 1023-k.

Self-contained: hardcodes all shapes; no sibling imports.
"""
import sys
import numpy as np

sys.path.insert(0, '/opt/trn_rl_repo')

C, L, FEAT = 64, 1024, 768
H = 80
SPAN = 4 * H            # 320
N_CORES = 8
CPC = C // N_CORES      # comments per core = 8
TOK = CPC * L           # tokens per core = 8192
KCH = FEAT // 128       # 6 contraction chunks
TBLK = 512              # projection token block (64 steps)
NTB = TOK // TBLK       # 16
GD = 8                  # gate-dir count
ROWBASE = [0, 80, 240, 160]   # i, f, o, g -> torch row offset

TOK_FP8 = False         # token/wih upload dtype switch

_compiled = None

# Warm the axon/jax platform at import time (device discovery is a
# one-time global cost; keep it out of the compute path).
try:
    import jax as _jax
    _jax.devices()
except Exception:
    pass


def _build():
    import concourse.bass as bass
    import concourse.tile as tile
    from concourse import bacc, mybir
    from contextlib import ExitStack

    f16, f32 = mybir.dt.float16, mybir.dt.float32
    tdt = mybir.dt.float8e4 if TOK_FP8 else f16
    ACT = mybir.ActivationFunctionType

    nc = bacc.Bacc("TRN2", target_bir_lowering=False, debug=False,
                   enable_asserts=False, num_devices=N_CORES)

    xt_d = nc.dram_tensor("xt", [KCH, 128, TOK], tdt, kind="ExternalInput").ap()
    wih_d = nc.dram_tensor("wih", [KCH, GD, 128, H], tdt,
                           kind="ExternalInput").ap()
    whh_d = nc.dram_tensor("whh", [GD, H, H], f16, kind="ExternalInput").ap()
    b_d = nc.dram_tensor("b", [H, GD], f32, kind="ExternalInput").ap()
    ht_d = nc.dram_tensor("ht", [H, L, 2 * CPC], f16,
                          kind="ExternalOutput").ap()

    with tile.TileContext(nc) as tc, ExitStack() as ctx:
        state = ctx.enter_context(tc.tile_pool(name="st", bufs=1))
        xpool = ctx.enter_context(tc.tile_pool(name="x", bufs=2))
        gpool = ctx.enter_context(tc.tile_pool(name="g", bufs=2))
        ppool = ctx.enter_context(tc.tile_pool(name="p", bufs=4, space="PSUM"))

        # --- persistent tiles ---
        wih = state.tile([128, KCH * GD * H], tdt, tag="wih")
        for k in range(KCH):
            for gd in range(GD):
                nc.sync.dma_start(wih[:, (k * GD + gd) * H:(k * GD + gd + 1) * H],
                                  wih_d[k, gd])
        whh = state.tile([H, GD * H], f16, tag="whh")
        for gd in range(GD):
            nc.sync.dma_start(whh[:, gd * H:(gd + 1) * H], whh_d[gd])
        bias = state.tile([H, GD], f32, tag="bias")
        nc.sync.dma_start(bias[:], b_d[:])

        xp = state.tile([H, L, 4, 2 * CPC], f16, tag="xp")
        ht = state.tile([H, L + 1, 2 * CPC], f16, tag="ht")
        cst = state.tile([H, 2 * CPC], f32, tag="c")
        nc.vector.memset(ht[:, 0, :], 0.0)
        nc.vector.memset(cst[:], 0.0)

        # --- phase 1: input projections ---
        for tb in range(NTB):
            xts = []
            for k in range(KCH):
                xtile = xpool.tile([128, TBLK], tdt, tag=f"x{k}")
                nc.sync.dma_start(xtile[:], xt_d[k, :, tb * TBLK:(tb + 1) * TBLK])
                xts.append(xtile)
            for g in range(4):
                for d in range(2):
                    gd = g * 2 + d
                    ps = ppool.tile([H, TBLK // CPC, CPC], f32, tag="ps")
                    for k in range(KCH):
                        nc.tensor.matmul(
                            ps[:], wih[:, (k * GD + gd) * H:(k * GD + gd + 1) * H],
                            xts[k][:], start=(k == 0), stop=(k == KCH - 1))
                    # xp[:, steps, g, d*8:(d+1)*8] = ps + b[gd]
                    nc.scalar.activation(
                        xp[:, tb * (TBLK // CPC):(tb + 1) * (TBLK // CPC),
                           g, d * CPC:(d + 1) * CPC],
                        ps[:], ACT.Identity, bias=bias[:, gd:gd + 1])

        # --- phase 2: coupled fwd/bwd recurrence (hardware loop) ---
        pg = ppool.tile([H, 4, 2 * CPC], f32, tag="pg")
        gates = gpool.tile([H, 4, 2 * CPC], f32, tag="gates")
        sg = gpool.tile([H, 4, 2 * CPC], f32, tag="sg")
        th = gpool.tile([H, 2 * CPC], f32, tag="th")
        u = gpool.tile([H, 2 * CPC], f32, tag="u")
        fc = gpool.tile([H, 2 * CPC], f32, tag="fc")
        with tc.For_i(0, L) as k:
            for g in range(4):
                for d in range(2):
                    gd = g * 2 + d
                    nc.tensor.matmul(
                        pg[:, g, d * CPC:(d + 1) * CPC],
                        whh[:, gd * H:(gd + 1) * H],
                        ht[:, k, d * CPC:(d + 1) * CPC],
                        start=True, stop=True)
            # z = pg + xp (fwd reads step k, bwd reads step 1023-k)
            nc.vector.tensor_add(gates[:, :, 0:CPC], pg[:, :, 0:CPC],
                                 xp[:, k, :, 0:CPC])
            nc.vector.tensor_add(gates[:, :, CPC:2 * CPC], pg[:, :, CPC:2 * CPC],
                                 xp[:, L - 1 - k, :, CPC:2 * CPC])
            nc.scalar.activation(sg[:, 0:3, :], gates[:, 0:3, :], ACT.Sigmoid)
            nc.scalar.activation(sg[:, 3, :], gates[:, 3, :], ACT.Tanh)
            # c = sig(f)*c + sig(i)*tanh(g)
            nc.vector.tensor_mul(u[:], sg[:, 0, :], sg[:, 3, :])
            nc.vector.tensor_mul(fc[:], sg[:, 1, :], cst[:])
            nc.vector.tensor_add(cst[:], fc[:], u[:])
            nc.scalar.activation(th[:], cst[:], ACT.Tanh)
            # h = sig(o)*tanh(c) -> f16 history (next step's moving operand)
            nc.vector.tensor_mul(ht[:, k + 1, :], sg[:, 2, :], th[:])

        # --- output ---
        nc.sync.dma_start(ht_d[:], ht[:, 1:L + 1, :])
    nc.compile()
    return nc


def _np_tdt():
    from concourse import mybir
    return mybir.dt.np(mybir.dt.float8e4) if TOK_FP8 else np.float16


def _pack_inputs(inp):
    tdt = _np_tdt()
    token = inp['token_embed']                                # [C, L, 768] f32

    wih_p = np.empty((KCH, GD, 128, H), np.float32)
    whh_p = np.empty((GD, H, H), np.float32)
    b_p = np.empty((H, GD), np.float32)
    for g in range(4):
        rb = ROWBASE[g]
        for d in range(2):
            Wih = inp['Wih_f'] if d == 0 else inp['Wih_b']    # [320, 768]
            Whh = inp['Whh_f'] if d == 0 else inp['Whh_b']    # [320, 80]
            bb = inp['b_f'] if d == 0 else inp['b_b']         # [320]
            gd = g * 2 + d
            wih_p[:, gd] = Wih[rb:rb + H].reshape(H, KCH, 128).transpose(1, 2, 0)
            whh_p[gd] = Whh[rb:rb + H].T
            b_p[:, gd] = bb[rb:rb + H]
    wih_p = wih_p.astype(tdt)
    whh_p = whh_p.astype(np.float16)

    in_maps = []
    for core in range(N_CORES):
        tk = token[core * CPC:(core + 1) * CPC]               # [8, 1024, 768]
        xt = np.ascontiguousarray(
            tk.transpose(1, 0, 2).reshape(TOK, KCH, 128).transpose(1, 2, 0)
        ).astype(tdt)                                         # [6, 128, 8192]
        in_maps.append(dict(xt=xt, wih=wih_p, whh=whh_p, b=b_p))
    return in_maps


def _sigmoid(z):
    out = np.empty_like(z)
    np.negative(z, out)
    np.exp(out, out)
    out += 1.0
    np.reciprocal(out, out)
    return out


def _lstm(xp, Whh, nh, reverse=False):
    """xp: [T, B, 4*nh] precomputed x @ Wih.T + b. Exact fp32 recurrence."""
    Ln, B, _ = xp.shape
    Wt = Whh.T.astype(np.float32)
    h = np.zeros((B, nh), np.float32)
    c = np.zeros((B, nh), np.float32)
    hs = np.empty((Ln, B, nh), np.float32)
    order = range(Ln - 1, -1, -1) if reverse else range(Ln)
    for t in order:
        z = xp[t] + h @ Wt
        i, f, g, o = (z[:, :nh], z[:, nh:2 * nh],
                      z[:, 2 * nh:3 * nh], z[:, 3 * nh:])
        c = _sigmoid(f) * c + _sigmoid(i) * np.tanh(g)
        h = _sigmoid(o) * np.tanh(c)
        hs[t] = h
    return hs


def _attn_pool(feats, vals, mask, W1, b1, W2, b2):
    s = np.maximum(feats @ W1 + b1, 0.0) @ W2 + b2
    s = np.where(mask[:, None], s, -1e9)
    ex = np.exp(s - s.max(0, keepdims=True))
    a = ex / ex.sum(0, keepdims=True)
    a = np.where(mask[:, None], a, 0.0)
    out = (a * vals).sum(0)
    return np.where(mask.any(), out, np.zeros_like(out))


def _gat(h, src, dst, emask, Wm, a_l, a_r, bias):
    An, K = h.shape[0], Wm.shape[0]
    hp = np.stack([h @ Wm[k] for k in range(K)], 1)          # [A, K, D]
    el = (hp * a_l[None]).sum(-1)
    er = (hp * a_r[None]).sum(-1)
    e = el[src] + er[dst]
    e = np.where(e > 0, e, 0.2 * e)
    e = np.where(emask[:, None], e, -1e9)
    m = np.full((An, K), -1e9, np.float32)
    np.maximum.at(m, dst, e)
    ex = np.where(emask[:, None], np.exp(e - m[dst]), 0.0)
    den = np.zeros((An, K), np.float32)
    np.add.at(den, dst, ex)
    alpha = ex / np.maximum(den[dst], 1e-9)
    out = np.zeros((An, K, hp.shape[2]), np.float32)
    np.add.at(out, dst, alpha[:, :, None] * hp[src])
    out = out + bias[None]
    out = np.where(out > 0, out, np.expm1(np.minimum(out, 0.0)))
    return out.reshape(An, -1)


def kernel(**inputs):
    global _compiled
    inp = {k: np.asarray(v) for k, v in inputs.items()}
    in_maps = _pack_inputs(inp)

    if _compiled is None:
        _compiled = _build()
    globals()['_last_in_maps'] = in_maps
    from concourse.bass_utils import run_bass_kernel_spmd
    import time as _time
    _t0 = _time.time()
    res = run_bass_kernel_spmd(_compiled, in_maps,
                               core_ids=list(range(N_CORES)))
    globals()['_last_exec_ns'] = res.exec_time_ns
    globals()['_last_dispatch_s'] = _time.time() - _t0

    # unpack hidden states: ht [80, 1024, 16] f16 per core
    hf = np.empty((C, L, H), np.float32)
    hb = np.empty((C, L, H), np.float32)
    for core in range(N_CORES):
        htc = res.results[core]["ht"].astype(np.float32)      # [80, 1024, 16]
        hf[core * CPC:(core + 1) * CPC] = htc[:, :, 0:CPC].transpose(2, 1, 0)
        hb[core * CPC:(core + 1) * CPC] = \
            htc[:, ::-1, CPC:2 * CPC].transpose(2, 1, 0)

    # ---- host: graph heads (fp32) ----
    A = inp['adu_spans'].shape[1]
    W_gat = inp['W_gat'].astype(np.float32)

    def span_rep(c, spans):
        i, j = spans[..., 0], spans[..., 1]
        return np.concatenate([hf[c][j] - hf[c][i - 1], hb[c][i] - hb[c][j + 1],
                               hf[c][i - 1], hb[c][j + 1]], -1)

    rows = []
    for c in range(C):
        cemb = span_rep(c, inp['comment_spans'][c])
        amask = inp['adu_masks'][c]
        adus = span_rep(c, inp['adu_spans'][c]) * amask[:, None]
        isrc, idst = inp['inner_src'][c], inp['inner_dst'][c]
        irel, imask = inp['inner_rel'][c], inp['inner_mask'][c]
        tsrc, tdst = inp['inter_src'][c], inp['inter_dst'][c]
        trel, tmask = inp['inter_rel'][c], inp['inter_mask'][c]
        srcs = [isrc, isrc, tdst, tdst]
        dsts = [idst, idst, tsrc, tsrc]
        masks = [imask & (irel == 0), imask & (irel == 1),
                 tmask & (trel == 0), tmask & (trel == 1)]
        z = np.stack([_gat(adus, srcs[m], dsts[m], masks[m], W_gat[m],
                           inp['a_l'][m], inp['a_r'][m], inp['b_gat'][m])
                      for m in range(4)])                     # [4, A, 768]
        w = np.tanh(z.reshape(4 * A, -1) @ inp['W_sem'] + inp['b_sem'])
        w = (w @ inp['q_sem']).reshape(4, A)
        w = (w * amask[None]).sum(1) / max(amask.sum(), 1)
        beta = np.exp(w - w.max())
        beta /= beta.sum()
        zfin = np.einsum('m,mad->ad', beta, z)
        adu_embeds = zfin @ inp['W_pred'] + inp['b_pred']
        feats = np.concatenate(
            [np.broadcast_to(cemb, (A, SPAN)), adu_embeds], -1)
        att_adu = _attn_pool(feats, adu_embeds, amask & inp['local_masks'][c],
                             inp['W_adu1'], inp['b_adu1'],
                             inp['W_adu2'], inp['b_adu2'])

        def pair(se, de, rel, me, W1, b1, W2, b2):
            onehot = np.stack([rel, 1 - rel], -1).astype(np.float32)
            pe = np.concatenate([adu_embeds[se], adu_embeds[de], onehot], -1)
            fp = np.concatenate(
                [np.broadcast_to(cemb, (pe.shape[0], SPAN)), pe], -1)
            return _attn_pool(fp, pe, me, W1, b1, W2, b2)

        att_inn = pair(isrc, idst, irel, imask, inp['W_inn1'], inp['b_inn1'],
                       inp['W_inn2'], inp['b_inn2'])
        att_int = pair(tdst, tsrc, trel, tmask, inp['W_int1'], inp['b_int1'],
                       inp['W_int2'], inp['b_int2'])
        rows.append(np.concatenate(
            [att_adu, att_inn, att_int, inp['info_scores'][c], cemb]))
    wo_ctx = np.stack(rows).astype(np.float32)                # [64, 1608]

    xpc = (wo_ctx @ inp['Wih_c'].T + inp['b_c'])[:, None, :]  # [64, 1, 800]
    hs = _lstm(xpc, inp['Whh_c'], 200)[:, 0, :]               # [64, 200]
    return np.concatenate([hs, wo_ctx], -1).astype(np.float32)
